# revision 1
# baseline (speedup 1.0000x reference)
"""Trainium2 Bass kernel for ClassFeatureMemoryBank proto-contrastive loss.

Computes: mean over N=1M rows of  logsumexp(f_hat @ P.T / T) - (f_hat @ P.T / T)[label]
where f_hat = f / max(||f||, eps), P = [150, 128] L2-normalized prototypes.

Strategy (data-parallel over 8 cores, ~125k rows each):
  - SWDGE cast-DMA: HBM fp32 -> SBUF bf16 (halves SBUF fabric traffic)
  - DVE fused scalar_tensor_tensor ops: sumsq (q), fold+sumexp, masked-pick
  - Newton rsqrt (bit-hack + 2 iters) for s = 1/||f||, batched over 4 supertiles
  - GPSIMD tensor_scalar: fs = f * s (bf16)
  - HWDGE xbar DMA transpose: fs -> fsT per 128x128 block
  - PE: logits = fsT.T @ protosT_pad  (bf16, fp32 PSUM)
  - ACT: exp((1/T) * logits) one strided op per supertile, bf16 out
  - Host: loss = mean(log(sumexp - n_pad_classes) - log(mexp)) in float64
"""
import sys
import os

sys.path.insert(0, "/opt/trn_rl_repo")

import numpy as np
import ml_dtypes
from contextlib import ExitStack

import concourse.bass as bass
import concourse.tile as tile
from concourse import bacc, mybir
from concourse.bass_utils import run_bass_kernel_spmd

F32 = mybir.dt.float32
BF16 = mybir.dt.bfloat16
I32 = mybir.dt.int32
ALU = mybir.AluOpType

N_CORES = 8
D = 128
C = 150
CP = 160          # padded classes (10 pad cols, protos=0 -> exp=1 each)
CSLOT = 256       # fp32 slot per tile in PSUM (1024B, half bank)
TEMP = 0.15
G = 8             # tiles per supertile
NB = 4            # supertiles per newton batch

# full-size problem constants
N_FULL = 1_000_000
ROWS_PER_CORE_FULL = 125_952   # = 128 * 984 = 1024 * 123; 8*this >= 1M

_NC_CACHE = {}


def build_nc(rows_per_core: int):
    assert rows_per_core % (128 * G) == 0
    n_tiles = rows_per_core // 128
    n_super = n_tiles // G

    nc = bacc.Bacc("TRN2", target_bir_lowering=False, debug=False)
    feat = nc.dram_tensor("features", [rows_per_core, D], F32,
                          kind="ExternalInput").ap()
    labelsf = nc.dram_tensor("labelsf", [128, n_tiles], F32,
                             kind="ExternalInput").ap()
    protosT = nc.dram_tensor("protosT", [128, CP], BF16,
                             kind="ExternalInput").ap()
    iota = nc.dram_tensor("iota", [128, CP], BF16, kind="ExternalInput").ap()

    out_sumexp = nc.dram_tensor("sumexp", [128, n_tiles], F32,
                                kind="ExternalOutput").ap()
    out_mexp = nc.dram_tensor("mexp", [128, n_tiles], F32,
                              kind="ExternalOutput").ap()

    fview = feat.rearrange("(s g p) d -> s p g d", p=128, g=G)

    with tile.TileContext(nc) as tc, ExitStack() as ctx:
        const = ctx.enter_context(tc.tile_pool(name="const", bufs=1))
        fpool = ctx.enter_context(tc.tile_pool(name="f", bufs=2 * NB + 2))
        fspool = ctx.enter_context(tc.tile_pool(name="fs", bufs=3))
        ftpool = ctx.enter_context(tc.tile_pool(name="fsT", bufs=3))
        epool = ctx.enter_context(tc.tile_pool(name="expb", bufs=3))
        qpool = ctx.enter_context(tc.tile_pool(name="q", bufs=3))
        spool = ctx.enter_context(tc.tile_pool(name="s", bufs=3))
        scpool = ctx.enter_context(tc.tile_pool(name="scratch", bufs=2))
        pspool = ctx.enter_context(tc.tile_pool(name="ps", bufs=2, space="PSUM"))

        # constants / persistent buffers
        protosT_sb = const.tile([128, CP], BF16)
        nc.sync.dma_start(protosT_sb[:], protosT[:, :])
        iota_sb = const.tile([128, CP], BF16)
        nc.sync.dma_start(iota_sb[:], iota[:, :])
        labelsf_sb = const.tile([128, n_tiles], F32)
        nc.sync.dma_start(labelsf_sb[:], labelsf[:, :])
        kmagic = const.tile([128, G * NB], I32)
        nc.vector.memset(kmagic[:], 0x5F3759DF)

        sumexp_buf = const.tile([128, n_tiles], F32)
        mexp_buf = const.tile([128, n_tiles], F32)

        def newton_rsqrt(s_t, q_t, w):
            """s = 1/sqrt(q), elementwise on [128, w]."""
            bs = scpool.tile([128, G * NB], I32, tag="nt_bs")
            nc.vector.tensor_scalar(bs[:, 0:w], q_t[:, 0:w].bitcast(I32), 1, None,
                                    ALU.logical_shift_right)
            y0 = scpool.tile([128, G * NB], I32, tag="nt_y0")
            nc.vector.tensor_tensor(y0[:, 0:w], kmagic[:, 0:w], bs[:, 0:w],
                                    ALU.subtract)
            y0f = y0[:, 0:w].bitcast(F32)
            t = scpool.tile([128, G * NB], F32, tag="nt_t")
            # iter 1:  t = (q * -0.5) * y;  t = t * y;  y = (t + 1.5) * y
            nc.vector.scalar_tensor_tensor(t[:, 0:w], q_t[:, 0:w], -0.5, y0f,
                                           ALU.mult, ALU.mult)
            nc.vector.tensor_tensor(t[:, 0:w], t[:, 0:w], y0f, ALU.mult)
            nc.vector.scalar_tensor_tensor(s_t[:, 0:w], t[:, 0:w], 1.5, y0f,
                                           ALU.add, ALU.mult)
            # iter 2
            nc.vector.scalar_tensor_tensor(t[:, 0:w], q_t[:, 0:w], -0.5, s_t[:, 0:w],
                                           ALU.mult, ALU.mult)
            nc.vector.tensor_tensor(t[:, 0:w], t[:, 0:w], s_t[:, 0:w], ALU.mult)
            nc.vector.scalar_tensor_tensor(s_t[:, 0:w], t[:, 0:w], 1.5, s_t[:, 0:w],
                                           ALU.add, ALU.mult)

        n_batches = (n_super + NB - 1) // NB
        for b in range(n_batches):
            sts = list(range(b * NB, min((b + 1) * NB, n_super)))
            w = len(sts) * G

            q_t = qpool.tile([128, G * NB], F32)
            f_tiles = {}
            for j, st in enumerate(sts):
                f = fpool.tile([128, G, D], BF16)
                nc.gpsimd.dma_start(f[:], fview[st])
                f_tiles[st] = f
                qsc = scpool.tile([128, D], BF16, tag="qsc")
                for g in range(G):
                    nc.vector.scalar_tensor_tensor(
                        qsc[:], f[:, g, :], 1.0, f[:, g, :],
                        ALU.mult, ALU.mult, q_t[:, j * G + g : j * G + g + 1])

            s_t = spool.tile([128, G * NB], F32)
            newton_rsqrt(s_t, q_t, w)

            for j, st in enumerate(sts):
                f = f_tiles[st]
                fs = fspool.tile([128, G, D], BF16)
                for g in range(G):
                    nc.gpsimd.tensor_scalar(
                        fs[:, g, :], f[:, g, :],
                        s_t[:, j * G + g : j * G + g + 1], None, ALU.mult)

                fsT = ftpool.tile([128, G, D], BF16)
                for g in range(G):
                    nc.sync.dma_start(fsT[:, g, :], fs[:, g, :], transpose=True)

                logits = pspool.tile([128, G, CSLOT], F32)
                for g in range(G):
                    nc.tensor.matmul(logits[:, g, 0:CP], fsT[:, g, :],
                                     protosT_sb[:], start=True, stop=True)

                expb = epool.tile([128, G, CP], BF16)
                nc.scalar.activation(expb[:], logits[:, :, 0:CP],
                                     mybir.ActivationFunctionType.Exp,
                                     bias=0.0, scale=1.0 / TEMP)

                fold = scpool.tile([128, CP // 2], BF16, tag="fold")
                psc = scpool.tile([128, CP], BF16, tag="psc")
                for g in range(G):
                    ti = st * G + g
                    nc.vector.scalar_tensor_tensor(
                        fold[:], expb[:, g, 0 : CP // 2], 1.0,
                        expb[:, g, CP // 2 : CP],
                        ALU.mult, ALU.add, sumexp_buf[:, ti : ti + 1])
                    nc.vector.scalar_tensor_tensor(
                        psc[:], iota_sb[:], labelsf_sb[:, ti : ti + 1],
                        expb[:, g, :],
                        ALU.is_equal, ALU.mult, mexp_buf[:, ti : ti + 1])

        nc.sync.dma_start(out_sumexp[:, :], sumexp_buf[:])
        nc.sync.dma_start(out_mexp[:, :], mexp_buf[:])

    nc.compile()
    return nc


def _get_nc(rows_per_core):
    if rows_per_core not in _NC_CACHE:
        _NC_CACHE[rows_per_core] = build_nc(rows_per_core)
    return _NC_CACHE[rows_per_core]


def _prep_core_inputs(features, labels, prototypes, rows_per_core):
    """Shard + host-side prep. Returns (in_maps, n_valid_per_core)."""
    n = features.shape[0]
    n_tiles = rows_per_core // 128

    protosT_pad = np.zeros((128, CP), dtype=np.float32)
    protosT_pad[:, :C] = prototypes.T.astype(np.float32)
    protosT_pad = protosT_pad.astype(ml_dtypes.bfloat16)
    iota_np = np.broadcast_to(np.arange(CP, dtype=np.float32), (128, CP))
    iota_np = np.ascontiguousarray(iota_np).astype(ml_dtypes.bfloat16)

    in_maps = []
    n_valid = []
    for c in range(N_CORES):
        lo = c * rows_per_core
        hi = min(n, lo + rows_per_core)
        valid = max(0, hi - lo)
        n_valid.append(valid)
        if valid == rows_per_core:
            fshard = features[lo:hi]
            lshard = labels[lo:hi]
        else:
            fshard = np.zeros((rows_per_core, D), dtype=np.float32)
            fshard[:, 0] = 1.0  # unit rows: q=1, harmless
            lshard = np.zeros(rows_per_core, dtype=np.int64)
            if valid > 0:
                fshard[:valid] = features[lo:hi]
                lshard[:valid] = labels[lo:hi]
        # labelsf[p, t] = label of row t*128 + p
        labelsf = np.ascontiguousarray(
            lshard.reshape(n_tiles, 128).T).astype(np.float32)
        in_maps.append({
            "features": np.ascontiguousarray(fshard),
            "labelsf": labelsf,
            "protosT": protosT_pad,
            "iota": iota_np,
        })
    return in_maps, n_valid


def run_cores(features, labels, prototypes, rows_per_core, trace=False):
    nc = _get_nc(rows_per_core)
    in_maps, n_valid = _prep_core_inputs(features, labels, prototypes,
                                         rows_per_core)
    res = run_bass_kernel_spmd(nc, in_maps, core_ids=list(range(N_CORES)),
                               trace=trace)
    return res, n_valid


def _reduce_host(res, n_valid, rows_per_core, n_total):
    n_tiles = rows_per_core // 128
    total = 0.0
    for c in range(N_CORES):
        valid = n_valid[c]
        if valid == 0:
            continue
        sumexp = res.results[c]["sumexp"].astype(np.float64)  # [128, n_tiles]
        mexp = res.results[c]["mexp"].astype(np.float64)
        # row index of (p, t) is t*128 + p
        p = np.arange(128)[:, None]
        t = np.arange(n_tiles)[None, :]
        mask = (t * 128 + p) < valid
        logz = np.log(sumexp[mask] - (CP - C))
        picked = np.log(mexp[mask])
        total += (logz - picked).sum()
    return np.float32(total / n_total)


def kernel(features, labels, prototypes):
    features = np.asarray(features, dtype=np.float32)
    labels = np.asarray(labels)
    prototypes = np.asarray(prototypes, dtype=np.float32)
    n = features.shape[0]
    if n == N_FULL:
        rows_per_core = ROWS_PER_CORE_FULL
    else:
        # smallest multiple of 128*G covering n/8
        per = (n + N_CORES - 1) // N_CORES
        rows_per_core = ((per + 128 * G - 1) // (128 * G)) * (128 * G)
    res, n_valid = run_cores(features, labels, prototypes, rows_per_core)
    return _reduce_host(res, n_valid, rows_per_core, n)


if __name__ == "__main__":
    # quick self-test with small n
    rng = np.random.default_rng(0)
    n = 8 * 128 * G * 2
    f = rng.normal(size=(n, D)).astype(np.float32)
    lab = rng.integers(0, C, size=n).astype(np.int64)
    p = rng.normal(size=(C, D)).astype(np.float32)
    p /= np.linalg.norm(p, axis=1, keepdims=True)
    got = kernel(f, lab, p)

    fh = f / np.maximum(np.linalg.norm(f, axis=1, keepdims=True), 1e-12)
    logits = fh @ p.T / TEMP
    m = logits.max(axis=1, keepdims=True)
    logz = np.log(np.exp(logits - m).sum(1)) + m[:, 0]
    picked = np.take_along_axis(logits, lab[:, None], axis=1)[:, 0]
    want = (logz - picked).mean()
    print("got:", got, "want:", want, "rel:", abs(got / want - 1))



# revision 2
# speedup vs baseline: 72.9608x; 72.9608x over previous
"""Trainium2 Bass kernel for ClassFeatureMemoryBank proto-contrastive loss (v2).

Computes: mean over N=1M rows of  logsumexp(f_hat @ P.T / T) - (f_hat @ P.T / T)[label]
where f_hat = f / max(||f||, eps), P = [150, 128] L2-normalized prototypes.

v2 strategy (data-parallel over 8 cores, ~125k rows each):
  - Host prep: shard features, pre-cast to bf16 and pre-transpose into
    [n_groups, 128(d), 512(r)] blocks -> single HWDGE DMA per group with
    1KB contiguous partition lines; no on-device transposes or cast-DMAs.
  - DVE: f2 = fT*fT (one op per group)
  - PE: q[r] = ones-matmul over f2 (4 tiny matmuls per group, row-major out)
        logits[r, c] = fT.T @ protosT (4 matmuls per group)
  - DVE: Newton rsqrt (bit-hack + 2 iters) on q batched over 8 groups,
        with the 1/TEMP factor folded into the last iteration
  - ACT: expb = Exp(logits * s/T) with per-partition scale AND fused
        accum_out -> sumexp column (one op per 128-row tile)
  - GPSIMD: psc = (iota == label) * expb, accum -> mexp column
  - Host: loss = mean(log(sumexp) - log(mexp)) in float64
"""
import sys

sys.path.insert(0, "/opt/trn_rl_repo")

import numpy as np
import ml_dtypes
from contextlib import ExitStack

import concourse.bass as bass
import concourse.tile as tile
from concourse import bacc, mybir
from concourse.bass_utils import run_bass_kernel_spmd

F32 = mybir.dt.float32
BF16 = mybir.dt.bfloat16
I32 = mybir.dt.int32
ALU = mybir.AluOpType

N_CORES = 8
D = 128
C = 150
TEMP = 0.15
GT = 4                 # tiles per group (PSUM-sized)
GW = GT * 128          # rows per group = 512
NB = 8                 # groups per newton batch

N_FULL = 1_000_000
ROWS_PER_CORE_FULL = 125_952   # = 512 * 246; 8*this >= 1M
G = GT                 # kept for test.py compat (rows_per_core % (128*G) == 0)

_NC_CACHE = {}


def build_nc(rows_per_core: int):
    assert rows_per_core % GW == 0
    n_groups = rows_per_core // GW
    n_tiles = rows_per_core // 128

    nc = bacc.Bacc("TRN2", target_bir_lowering=False, debug=False)
    fT = nc.dram_tensor("fT", [n_groups, 128, GW], BF16,
                        kind="ExternalInput").ap()
    labelsf = nc.dram_tensor("labelsf", [128, n_tiles], F32,
                             kind="ExternalInput").ap()
    protosT = nc.dram_tensor("protosT", [128, C], BF16,
                             kind="ExternalInput").ap()
    iota = nc.dram_tensor("iota", [128, C], BF16, kind="ExternalInput").ap()

    out_sumexp = nc.dram_tensor("sumexp", [128, n_tiles], F32,
                                kind="ExternalOutput").ap()
    out_mexp = nc.dram_tensor("mexp", [128, n_tiles], F32,
                              kind="ExternalOutput").ap()

    with tile.TileContext(nc) as tc, ExitStack() as ctx:
        const = ctx.enter_context(tc.tile_pool(name="const", bufs=1))
        fpool = ctx.enter_context(tc.tile_pool(name="f", bufs=NB + 3))
        f2pool = ctx.enter_context(tc.tile_pool(name="f2", bufs=3))
        epool = ctx.enter_context(tc.tile_pool(name="expb", bufs=3))
        spool = ctx.enter_context(tc.tile_pool(name="s", bufs=2))
        scpool = ctx.enter_context(tc.tile_pool(name="scratch", bufs=2))
        qpsum = ctx.enter_context(tc.tile_pool(name="qp", bufs=2, space="PSUM"))
        lpsum = ctx.enter_context(tc.tile_pool(name="lp", bufs=3, space="PSUM"))

        protosT_sb = const.tile([128, C], BF16)
        nc.sync.dma_start(protosT_sb[:], protosT[:, :])
        iota_sb = const.tile([128, C], BF16)
        nc.sync.dma_start(iota_sb[:], iota[:, :])
        labelsf_sb = const.tile([128, n_tiles], F32)
        nc.sync.dma_start(labelsf_sb[:], labelsf[:, :])
        ones_sb = const.tile([128, 1], BF16)
        nc.vector.memset(ones_sb[:], 1.0)
        kmagic = const.tile([128, GT * NB], I32)
        nc.vector.memset(kmagic[:], 0x5F3759DF)

        sumexp_buf = const.tile([128, n_tiles], F32)
        mexp_buf = const.tile([128, n_tiles], F32)

        def newton_rsqrt_scaled(sT_t, q_ps, w):
            """sT = (1/sqrt(q)) / TEMP, elementwise on [128, w]. q in PSUM."""
            q_sb = scpool.tile([128, GT * NB], F32, tag="nt_q")
            nc.scalar.copy(q_sb[:, 0:w], q_ps[:, 0:w])
            bs = scpool.tile([128, GT * NB], I32, tag="nt_bs")
            nc.vector.tensor_scalar(bs[:, 0:w], q_sb[:, 0:w].bitcast(I32), 1,
                                    None, ALU.logical_shift_right)
            y0 = scpool.tile([128, GT * NB], I32, tag="nt_y0")
            nc.vector.tensor_tensor(y0[:, 0:w], kmagic[:, 0:w], bs[:, 0:w],
                                    ALU.subtract)
            y0f = y0[:, 0:w].bitcast(F32)
            t = scpool.tile([128, GT * NB], F32, tag="nt_t")
            # iter 1:  t = (q * -0.5) * y;  t = t * y;  y1 = (t + 1.5) * y
            y1 = scpool.tile([128, GT * NB], F32, tag="nt_y1")
            nc.vector.scalar_tensor_tensor(t[:, 0:w], q_sb[:, 0:w], -0.5, y0f,
                                           ALU.mult, ALU.mult)
            nc.vector.tensor_tensor(t[:, 0:w], t[:, 0:w], y0f, ALU.mult)
            nc.vector.scalar_tensor_tensor(y1[:, 0:w], t[:, 0:w], 1.5, y0f,
                                           ALU.add, ALU.mult)
            # iter 2 with 1/TEMP folded:
            #   t = (q * -0.5) * y1           [= -q*y1/2]
            #   t = (t * 1/T) * y1            [= -q*y1^2/(2T)]
            #   sT = (t + 1.5/T) * y1         [= (1.5 - q*y1^2/2) * y1 / T]
            nc.vector.scalar_tensor_tensor(t[:, 0:w], q_sb[:, 0:w], -0.5,
                                           y1[:, 0:w], ALU.mult, ALU.mult)
            nc.vector.scalar_tensor_tensor(t[:, 0:w], t[:, 0:w], 1.0 / TEMP,
                                           y1[:, 0:w], ALU.mult, ALU.mult)
            nc.vector.scalar_tensor_tensor(sT_t[:, 0:w], t[:, 0:w],
                                           1.5 / TEMP, y1[:, 0:w],
                                           ALU.add, ALU.mult)

        n_batches = (n_groups + NB - 1) // NB
        for b in range(n_batches):
            gis = list(range(b * NB, min((b + 1) * NB, n_groups)))
            w = len(gis) * GT

            # phase A: load + f2 + q matmuls
            q_ps = qpsum.tile([128, GT * NB], F32)
            f_tiles = {}
            for j, gi in enumerate(gis):
                ft = fpool.tile([128, GT, D], BF16)
                nc.sync.dma_start(ft[:], fT[gi].rearrange("p (g r) -> p g r",
                                                          g=GT))
                f_tiles[gi] = ft
                f2 = f2pool.tile([128, GT, D], BF16)
                nc.gpsimd.tensor_tensor(f2[:], ft[:], ft[:], ALU.mult)
                for g in range(GT):
                    col = j * GT + g
                    nc.tensor.matmul(q_ps[:, col:col + 1], f2[:, g, :],
                                     ones_sb[:], start=True, stop=True)

            sT = spool.tile([128, GT * NB], F32)
            newton_rsqrt_scaled(sT, q_ps, w)

            # phase B: logits matmul -> exp(scale)+sumexp -> label pick
            psc = scpool.tile([128, C], BF16, tag="psc")
            for j, gi in enumerate(gis):
                ft = f_tiles[gi]
                logits = lpsum.tile([128, GT, 256], F32)
                for g in range(GT):
                    nc.tensor.matmul(logits[:, g, 0:C], ft[:, g, :],
                                     protosT_sb[:], start=True, stop=True)
                expb = epool.tile([128, GT, C], BF16)
                for g in range(GT):
                    ti = gi * GT + g
                    col = j * GT + g
                    nc.scalar.activation(expb[:, g, :], logits[:, g, 0:C],
                                         mybir.ActivationFunctionType.Exp,
                                         bias=0.0, scale=sT[:, col:col + 1],
                                         accum_out=sumexp_buf[:, ti:ti + 1])
                    nc.vector.scalar_tensor_tensor(
                        psc[:], iota_sb[:], labelsf_sb[:, ti:ti + 1],
                        expb[:, g, :], ALU.is_equal, ALU.mult,
                        mexp_buf[:, ti:ti + 1])

        nc.sync.dma_start(out_sumexp[:, :], sumexp_buf[:])
        nc.sync.dma_start(out_mexp[:, :], mexp_buf[:])

    nc.compile()
    return nc


def _get_nc(rows_per_core):
    if rows_per_core not in _NC_CACHE:
        _NC_CACHE[rows_per_core] = build_nc(rows_per_core)
    return _NC_CACHE[rows_per_core]


def _prep_core_inputs(features, labels, prototypes, rows_per_core):
    """Shard + host-side layout prep. Returns (in_maps, n_valid_per_core)."""
    n = features.shape[0]
    n_tiles = rows_per_core // 128
    n_groups = rows_per_core // GW

    protosT_np = np.ascontiguousarray(
        prototypes.T.astype(np.float32)).astype(ml_dtypes.bfloat16)
    iota_np = np.ascontiguousarray(
        np.broadcast_to(np.arange(C, dtype=np.float32), (128, C))
    ).astype(ml_dtypes.bfloat16)

    in_maps = []
    n_valid = []
    for c in range(N_CORES):
        lo = c * rows_per_core
        hi = min(n, lo + rows_per_core)
        valid = max(0, hi - lo)
        n_valid.append(valid)
        if valid == rows_per_core:
            fshard = features[lo:hi]
            lshard = labels[lo:hi]
        else:
            fshard = np.zeros((rows_per_core, D), dtype=np.float32)
            fshard[:, 0] = 1.0  # unit rows: q=1, harmless
            lshard = np.zeros(rows_per_core, dtype=np.int64)
            if valid > 0:
                fshard[:valid] = features[lo:hi]
                lshard[:valid] = labels[lo:hi]
        # fT[gi, d, r] = f[gi*GW + r, d], bf16
        fT = np.ascontiguousarray(
            fshard.reshape(n_groups, GW, D).transpose(0, 2, 1)
        ).astype(ml_dtypes.bfloat16)
        # labelsf[p, t] = label of row t*128 + p
        labelsf = np.ascontiguousarray(
            lshard.reshape(n_tiles, 128).T).astype(np.float32)
        in_maps.append({
            "fT": fT,
            "labelsf": labelsf,
            "protosT": protosT_np,
            "iota": iota_np,
        })
    return in_maps, n_valid


def run_cores(features, labels, prototypes, rows_per_core, trace=False):
    nc = _get_nc(rows_per_core)
    in_maps, n_valid = _prep_core_inputs(features, labels, prototypes,
                                         rows_per_core)
    res = run_bass_kernel_spmd(nc, in_maps, core_ids=list(range(N_CORES)),
                               trace=trace)
    return res, n_valid


def _reduce_host(res, n_valid, rows_per_core, n_total):
    n_tiles = rows_per_core // 128
    total = 0.0
    for c in range(N_CORES):
        valid = n_valid[c]
        if valid == 0:
            continue
        sumexp = res.results[c]["sumexp"].astype(np.float64)  # [128, n_tiles]
        mexp = res.results[c]["mexp"].astype(np.float64)
        # row index of (p, t) is t*128 + p
        p = np.arange(128)[:, None]
        t = np.arange(n_tiles)[None, :]
        mask = (t * 128 + p) < valid
        logz = np.log(sumexp[mask])
        picked = np.log(mexp[mask])
        total += (logz - picked).sum()
    return np.float32(total / n_total)


def kernel(features, labels, prototypes):
    features = np.asarray(features, dtype=np.float32)
    labels = np.asarray(labels)
    prototypes = np.asarray(prototypes, dtype=np.float32)
    n = features.shape[0]
    if n == N_FULL:
        rows_per_core = ROWS_PER_CORE_FULL
    else:
        # smallest multiple of GW covering n/8
        per = (n + N_CORES - 1) // N_CORES
        rows_per_core = ((per + GW - 1) // GW) * GW
    res, n_valid = run_cores(features, labels, prototypes, rows_per_core)
    return _reduce_host(res, n_valid, rows_per_core, n)


if __name__ == "__main__":
    # quick self-test with small n
    rng = np.random.default_rng(0)
    n = 8 * GW * 17 + 300   # exercises padding path too
    f = rng.normal(size=(n, D)).astype(np.float32)
    lab = rng.integers(0, C, size=n).astype(np.int64)
    p = rng.normal(size=(C, D)).astype(np.float32)
    p /= np.linalg.norm(p, axis=1, keepdims=True)
    got = kernel(f, lab, p)

    fh = f / np.maximum(np.linalg.norm(f, axis=1, keepdims=True), 1e-12)
    logits = fh @ p.T / TEMP
    m = logits.max(axis=1, keepdims=True)
    logz = np.log(np.exp(logits - m).sum(1)) + m[:, 0]
    picked = np.take_along_axis(logits, lab[:, None], axis=1)[:, 0]
    want = (logz - picked).mean()
    print("got:", got, "want:", want, "rel:", abs(got / want - 1))
